# revision 29
# baseline (speedup 1.0000x reference)
"""Trainium2 Bass kernel for nn_DegreePrediction.

Math (N=96):
  wtc = weights_t with [0,0] and [1,1] set to 0   (weights_t_fix provably
        never contributes: the mask picks wtf only at (0,0),(1,1), which
        __init__ zeroes)
  l2  = x * wtc                                   (96,96)
  wrc = weights_r with the 16 entries at {0,1}^4 replaced by constants
  y[k] = sum_{i,j,l} l2[i,j] * wrc[i,j,k,l]

Memory-bound: weights_r is 96^4 fp32 = 340 MB. Sharded over the first axis
i across 8 NeuronCores (12 i-values -> 42.5 MB per core, a contiguous DRAM
slab). Per core, the slab is viewed as (1152 rows = (i,j) pairs, 9216 = k*l)
and processed in 9 chunks of 128 rows:
  - DMA chunk (128 part x 36 KB contiguous per part) into SBUF (triple buf)
  - DVE reduce over l: (128, 96k, 96l) -> s(128, 96)
  - PE matmul accumulates into PSUM: y(1,96) += l2_col(128,1).T @ s(128,96)
Row p of chunk c is flat row p*9+c, so l2 columns come from a plain
(128, 9) reshape of the slab's l2 values.

Host side only shards/reshapes inputs, applies the *constant* substitutions
(zeroing wt[0,0], wt[1,1]; writing the 16 fixed weights_r values - these are
data-independent), and sums the 8 partial outputs.
"""

import numpy as np

_NC_CACHE = {}

N = 96
NCORES = 8
SLAB = N // NCORES          # 12 i-values per core
ROWS = SLAB * N             # 1152 (i,j) rows per core
CHUNKS = 9                  # 1152 = 128 * 9
KL = N * N                  # 9216

# fixed values written into weights_r at the 16 indices {0,1}^4
_FIX = np.zeros((2, 2, 2, 2), dtype=np.float32)
for _idx in ((0, 0, 0, 0), (0, 1, 0, 0), (1, 0, 1, 1), (1, 1, 1, 1)):
    _FIX[_idx] = 1.0


def _strip_redundant_waits(nc):
    """Drop sync waits already implied by happens-before.

    walrus's instruction encodings here have very few sync-wait slots, and
    Tile's sem assignment is explicitly not transitively minimal.  We compute
    a conservative happens-before closure and delete waits it implies:

    * compute engines execute serially, so an instruction inherits the
      knowledge of its engine predecessor (incl. that predecessor's sem
      increments, which fire at its completion);
    * DMA completions are asynchronous and their 16 per-SDMA-engine sem
      increments interleave between queue neighbours, so DMAs get NO implicit
      stream predecessor: their knowledge is only their own waits' imports;
    * waiting (S >= v) imports the completion-knowledge of the instruction
      whose increment brought S to v (well-defined because each sem here is
      incremented by a single stream; sems with mixed writers or decrements
      are poisoned and never used for dropping).
    """
    import concourse.mybir as mybir

    for f in nc.m.functions:
        # straight-line program: process blocks in listed order as one stream
        insts = [i for blk in f.blocks for i in blk.instructions]
        if True:
            n = len(insts)
            is_dma = [isinstance(i, mybir.InstDMACopy) for i in insts]
            # stream key: dispatch stream for compute; DMAs complete async
            # and get a unique stream each (no implicit predecessor)
            stream = [
                ("dma", idx) if d else ("eng", i.engine)
                for idx, (i, d) in enumerate(zip(insts, is_dma))
            ]
            # incrementer lists per sem: (inst_idx, cumulative_after)
            producers = {}
            poisoned = set()
            sem_stream = {}
            for idx, inst in enumerate(insts):
                si = getattr(inst, "sync_info", None)
                if not si:
                    continue
                for u in si.on_update or []:
                    sid = u.id
                    if u.update_mode == "sem-inc":
                        delta = 1
                    elif u.update_mode == "sem-add-imm":
                        delta = u.update_value or 0
                    else:
                        poisoned.add(sid)
                        continue
                    # producers of one sem must form a totally-ordered stream:
                    # either a single compute engine (serial execution), or
                    # DMAs of a single queue (ordered inductively: every kept
                    # or dropped-as-implied wait preserves issue order)
                    skey = (
                        ("dmaq", inst.engine)
                        if is_dma[idx]
                        else ("eng", inst.engine)
                    )
                    if sid in sem_stream and sem_stream[sid] != skey:
                        poisoned.add(sid)
                    sem_stream.setdefault(sid, skey)
                    lst = producers.setdefault(sid, [])
                    prev = lst[-1][1] if lst else 0
                    lst.append((idx, prev + delta))

            def producer_of(sid, v):
                # first instruction whose increment brings sem sid to >= v
                for idx, cum in producers.get(sid, []):
                    if cum >= v:
                        return idx
                return None

            def merge(dst, src):
                for s, v in src.items():
                    if dst.get(s, 0) < v:
                        dst[s] = v

            # completion-knowledge per instruction (guaranteed sem floors
            # once this instruction has COMPLETED)
            know_done = [None] * n
            last_in_stream = {}
            for idx, inst in enumerate(insts):
                si = getattr(inst, "sync_info", None)
                k = {}
                pred = last_in_stream.get(stream[idx])
                if pred is not None:
                    merge(k, know_done[pred])
                if is_dma[idx]:
                    # also ordered after the dispatching engine's stream?
                    # (issue order only; completions async) - skip.
                    pass
                waits = list(si.on_wait) if si and si.on_wait else []
                for w in waits:
                    if w.wait_mode != "sem-ge-imm" or w.wait_reg is not None:
                        continue
                    v = w.wait_value or 0
                    if k.get(w.id, 0) >= v:
                        continue
                    k[w.id] = v
                    if w.id not in poisoned:
                        p = producer_of(w.id, v)
                        if p is not None and know_done[p] is not None:
                            merge(k, know_done[p])
                # now try dropping waits implied without themselves
                if waits:
                    kept = list(waits)
                    changed = True
                    while changed:
                        changed = False
                        for w in list(kept):
                            if w.wait_mode != "sem-ge-imm" or w.wait_reg is not None:
                                continue
                            if w.id in poisoned:
                                continue
                            base = {}
                            if pred is not None:
                                merge(base, know_done[pred])
                            for o in kept:
                                if o is w:
                                    continue
                                if (
                                    o.wait_mode != "sem-ge-imm"
                                    or o.wait_reg is not None
                                ):
                                    continue
                                ov = o.wait_value or 0
                                base[o.id] = max(base.get(o.id, 0), ov)
                                if o.id not in poisoned:
                                    p = producer_of(o.id, ov)
                                    if p is not None and know_done[p] is not None:
                                        merge(base, know_done[p])
                            if base.get(w.id, 0) >= (w.wait_value or 0):
                                kept.remove(w)
                                changed = True
                    if len(kept) != len(waits):
                        si.on_wait = kept
                # add own increments to completion knowledge
                if si:
                    for u in si.on_update or []:
                        if u.id in poisoned:
                            continue
                        if u.update_mode == "sem-inc":
                            delta = 1
                        elif u.update_mode == "sem-add-imm":
                            delta = u.update_value or 0
                        else:
                            continue
                        # cumulative value after this inst's increment
                        for pidx, cum in producers.get(u.id, []):
                            if pidx == idx:
                                if k.get(u.id, 0) < cum:
                                    k[u.id] = cum
                                break
                know_done[idx] = k
                last_in_stream[stream[idx]] = idx


def _build_nc(passes=1, dma="gpsimd", halves=2):
    import concourse.bass as bass
    import concourse.mybir as mybir
    import concourse.tile as tile

    f32 = mybir.dt.float32
    nc = bass.Bass(
        "TRN2", target_bir_lowering=False, debug=False, num_devices=NCORES
    )
    wr_d = nc.dram_tensor("wr", (128, CHUNKS, KL), f32, kind="ExternalInput")
    # x-slab and wt-slab packed side by side -> one DMA, one sem for the mul
    xw_d = nc.dram_tensor("xw", (128, 2 * CHUNKS), f32, kind="ExternalInput")
    y_d = nc.dram_tensor("y", (1, N), f32, kind="ExternalOutput")

    # Sub-chunks: halves*CHUNKS DMAs of (128, KL/halves). The redundant-wait
    # stripper (below) keeps every DMA at <=1 sync wait and every compute
    # instruction at <=1-2, which walrus's static encodings require.
    HK = KL // halves
    HN = N // halves
    dma_eng = {"gpsimd": nc.gpsimd, "sync": nc.sync, "scalar": nc.scalar}[dma]
    # big-pool bufs must EQUAL the 8-lane DMA-sem round-robin period so a
    # slot's previous writer is exactly its lane predecessor; then the
    # consumer-chain (WAR) wait implies the lane-FIFO wait and the stripper
    # can keep every DMA at one sync wait.
    nbufs = 8
    assert (168 * 1024) // (HK * 4) >= nbufs, "chunk too large for 8 buffers"
    with tile.TileContext(nc) as tc:
        with (
            tc.tile_pool(name="big", bufs=nbufs) as big,
            tc.tile_pool(name="small", bufs=1) as small,
            tc.tile_pool(name="spool", bufs=halves * CHUNKS) as spool,
            tc.tile_pool(name="psum", bufs=1, space=bass.MemorySpace.PSUM) as pp,
        ):
            xw = small.tile([128, 2 * CHUNKS], f32, tag="xw")
            l2 = small.tile([128, CHUNKS], f32, tag="l2")
            dma_eng.dma_start(xw[:], xw_d[:])
            nc.vector.tensor_mul(l2[:], xw[:, :CHUNKS], xw[:, CHUNKS:])

            yps = [
                pp.tile([1, HN], f32, tag=f"yp{h}", name=f"yp{h}")
                for h in range(halves)
            ]
            for m in range(passes):
                for r in range(CHUNKS):
                    for h in range(halves):
                        wt = big.tile([128, HK], f32, tag="wr")
                        dma_eng.dma_start(
                            wt[:], wr_d[:, r, h * HK : (h + 1) * HK]
                        )
                        st = spool.tile([128, HN], f32, tag="s")
                        nc.vector.reduce_sum(
                            st[:],
                            wt[:].rearrange("p (k l) -> p k l", l=N),
                            axis=mybir.AxisListType.X,
                        )
                        # matmuls only on the last pass: earlier (timing-only)
                        # passes re-read the same data, so the result is
                        # unchanged and per-pass DMA/DVE work is identical
                        if m == passes - 1:
                            nc.tensor.matmul(
                                yps[h][:],
                                l2[:, r : r + 1],
                                st[:],
                                start=(r == 0),
                                stop=(r == CHUNKS - 1),
                            )
            ysb = small.tile([1, N], f32, tag="ysb")
            for h in range(halves):
                nc.vector.tensor_copy(ysb[:, h * HN : (h + 1) * HN], yps[h][:])
            dma_eng.dma_start(y_d[:], ysb[:])
    _strip_redundant_waits(nc)
    return nc


def get_nc(passes=1, dma="sync", halves=4):
    key = ("nc", passes, dma, halves)
    if key not in _NC_CACHE:
        _NC_CACHE[key] = _build_nc(passes, dma, halves)
    return _NC_CACHE[key]


def make_in_maps(x, weights_t, weights_r):
    """Shard full inputs into the 8 per-core input maps."""
    x = np.ascontiguousarray(np.asarray(x, dtype=np.float32))
    wt = np.array(np.asarray(weights_t, dtype=np.float32), copy=True)
    wr = np.asarray(weights_r, dtype=np.float32)
    # constant substitutions from the module's __init__/mask (data-independent)
    wt[0, 0] = 0.0
    wt[1, 1] = 0.0
    in_maps = []
    for c in range(NCORES):
        sl = slice(c * SLAB, (c + 1) * SLAB)
        wr_slab = wr[sl]
        if c == 0:
            wr_slab = wr_slab.copy()
            wr_slab[0:2, 0:2, 0:2, 0:2] = _FIX
        xw = np.concatenate(
            [
                np.ascontiguousarray(x[sl]).reshape(128, CHUNKS),
                np.ascontiguousarray(wt[sl]).reshape(128, CHUNKS),
            ],
            axis=1,
        )
        in_maps.append(
            {
                "wr": np.ascontiguousarray(wr_slab).reshape(128, CHUNKS, KL),
                "xw": np.ascontiguousarray(xw),
            }
        )
    return in_maps


def kernel(x, weights_t_fix, weights_t, weights_r, _want_results=False):
    from concourse.bass_utils import run_bass_kernel_spmd

    in_maps = make_in_maps(x, weights_t, weights_r)
    nc = get_nc()
    res = run_bass_kernel_spmd(nc, in_maps, core_ids=list(range(NCORES)))
    y = np.zeros(N, dtype=np.float64)
    for r in res.results:
        y += r["y"].reshape(N).astype(np.float64)
    out = y.astype(np.float32)
    if _want_results:
        return out, res
    return out
